# revision 28
# baseline (speedup 1.0000x reference)
"""ExternalAttention kernel for Trainium2 (8 NeuronCores, batch-parallel).

Math (collapsed from the reference nn.Module):
  q = (poi_data @ wq1 + bq1)[:, 0] @ wq2 + bq2            # [512], shared
  per head h: wkq[:, h] = wk[:, 64h:64h+64] @ q[64h:64h+64] # [512, 8]
  scores = x @ wkq  (+ const per head -- cancels in softmax)
  A = softmax(scores / 8, axis=L)
  xa[h, :] = sum_l A[l, h] * x[l, :]                       # [8, 512]
  V[64h:64h+64] = xa[h] @ wv[:, 64h:64h+64]                # [512]
  row = (V / Z) @ wo + (bv @ wo + bo)                      # [512]
  out[b, l, :] = row_b  for every l.

Design (v6): x is cast to fp8(e4m3) on the host (rel-err gate is 2e-2,
measured ~5e-3).  Two-phase stream separated by a no_sync scheduler
barrier:

  Phase 1 (XBAR DMA-transposes, SP+Act): score tiles st[p,j,t](u16) =
    xpair[t, 128j+p]; fp8 d = 256j+2p+b.  Score matmuls contract
    (j, b) slices against a host-permuted wkq (plain-loaded first on
    SP; its single fence link delays only the first transpose), exp()
    emits tiny f16 p tiles [128, 32] that all stay live, and the
    softmax denominator accumulates on PE via a ones-vector matmul.

  Phase 2 (plain DMAs, SP+Act+Pool): raw fp8 row loads feeding the xa
    matmuls (lhsT = x chunk, rhs = p) plus the f16 epilogue weights.
    The Z-normalization prep is emitted right after the barrier so it
    overlaps the phase-2 loads.

The phases exist because the tile scheduler completion-fences
DmaTransposeAnt against InstDMACopy in both directions (any
engine/tensor), and transpose-loaded weights consumed as f16 matmul
operands mis-execute on the neuronx-cc/PJRT path; the barrier keeps
the scheduler from re-interleaving the classes.  The write tail
broadcasts the row from a [128, 512] f16 SBUF tile via stride-0
source APs on all three queues.  PSUM stays f32; f16 output rows are
upcast on the host.
"""

import os
import sys

import numpy as np

for _p in ("/opt/trn_rl_repo", "/opt/pypackages"):
    if os.path.isdir(_p) and _p not in sys.path:
        sys.path.append(_p)

import ml_dtypes

B, L, D = 8, 8192, 512
H, DH = 8, 64
P = 128
SCALE = 1.0 / np.sqrt(DH)  # 0.125
N_CORES = 8
BCHUNK = 8   # chunks per exp batch
NBATCH = 8   # p batches

# phase 1: (engine, macro0, n_macros), 1 macro = 512 tokens
T_INSTS = [
    ("sp", 0, 2), ("act", 2, 2), ("sp", 4, 4),
    ("act", 8, 4), ("sp", 12, 4),
]
# phase 2: plain fp8 row loads, (engine, macro0, n_macros)
R_INSTS = [
    ("pool", 0, 2), ("sp", 2, 2), ("act", 4, 2), ("pool", 6, 2),
    ("sp", 8, 2), ("act", 10, 2), ("pool", 12, 2), ("sp", 14, 2),
]
W_ENG = {"wv": "act", "wo": "pool", "bo2": "sp"}
# write tail: (engine, row0, nrows)
W_INSTS = [
    ("sp", 0, 1024), ("sp", 1024, 1024), ("sp", 2048, 768),
    ("act", 2816, 1024), ("act", 3840, 1024), ("act", 4864, 640),
    ("pool", 5504, 1024), ("pool", 6528, 1024), ("pool", 7552, 640),
]

_CACHE = {}


def _build_bass():
    import concourse.bass as bass
    import concourse.tile as tile
    from concourse import mybir
    from concourse.bacc import Bacc

    f32 = mybir.dt.float32
    f16 = mybir.dt.float16
    f8 = mybir.dt.float8e4
    u16 = mybir.dt.uint16
    ts = bass.ts

    nc = Bacc(num_swdge_queues=4)
    x_d = nc.dram_tensor("x", [L, D], f8, kind="ExternalInput")
    wkq_d = nc.dram_tensor("wkq", [P, 32], f16, kind="ExternalInput")
    wv_d = nc.dram_tensor("wv", [P, 4, D], f16, kind="ExternalInput")
    wo_d = nc.dram_tensor("wo", [P, 4, D], f16, kind="ExternalInput")
    bo2_d = nc.dram_tensor("bo2", [P, 4], f16, kind="ExternalInput")
    row_d = nc.dram_tensor("row_scratch", [1, D], f16)
    out_d = nc.dram_tensor("out", [L, D], f16, kind="ExternalOutput")

    xu = x_d.bitcast(u16)  # [L, 256] pairs along d

    with tile.TileContext(nc) as tc:
        with (
            tc.tile_pool(name="consts", bufs=1) as consts,
            tc.tile_pool(name="xs", bufs=2) as xsp,
            tc.tile_pool(name="xr", bufs=6) as xrp,
            tc.tile_pool(name="pp", bufs=NBATCH) as ppp,
            tc.tile_pool(name="epi", bufs=1) as epi,
        ):
            eng = {"sp": nc.sync, "act": nc.scalar, "pool": nc.gpsimd}

            # wkq plain-loaded FIRST on SP (Act is busy with the exp
            # table load); only the first transpose fences behind it.
            wkq_sb = consts.tile([P, 2, 2, H], f16)
            nc.sync.dma_start(
                wkq_sb.rearrange("p j b h -> p (j b h)"), wkq_d[:, :]
            )
            ones_sb = consts.tile([P, 1], f16)
            nc.vector.memset(ones_sb, 1.0)
            warm = consts.tile([1, 8], f32)
            nc.vector.memset(warm, 0.0)
            warm_o = consts.tile([1, 8], f16)
            nc.scalar.activation(
                warm_o, warm, mybir.ActivationFunctionType.Exp, scale=1.0
            )

            wv_sb = consts.tile([P, 4, D], f16)
            wo_sb = consts.tile([P, 4, D], f16)
            bo2_sb = consts.tile([P, 4], f16)

            p_tiles = []

            with (
                tc.tile_pool(name="ps_acc", bufs=1, space="PSUM") as ps_acc,
                tc.tile_pool(name="ps_s", bufs=2, space="PSUM") as ps_s,
            ):
                z_ps = ps_acc.tile([1, 8 * BCHUNK], f32, name="zz", tag="zz")
                xa_ps = [
                    ps_acc.tile([P, H], f32, name=f"xa{k}", tag=f"xa{k}")
                    for k in range(4)
                ]

                # ---- phase 1: transposes, scores, exp, Z ----
                for e, m0, nm in T_INSTS:
                    tok0, ntok = m0 * 512, nm * 512
                    st = xsp.tile([P, 2, ntok], u16)
                    eng[e].dma_start(
                        st, xu[tok0 : tok0 + ntok, :], transpose=True
                    )
                    s8 = st.bitcast(f8).rearrange(
                        "p j (t b) -> p j t b", b=2
                    )
                    s_ps = None
                    for c in range(4 * nm):
                        if c % BCHUNK == 0:
                            s_ps = ps_s.tile([P, 8 * BCHUNK], f32)
                        col = 8 * (c % BCHUNK)
                        i = 0
                        for j in range(2):
                            for bb in range(2):
                                nc.tensor.matmul(
                                    s_ps[:, col : col + 8],
                                    s8[:, j, c * P : (c + 1) * P, bb],
                                    wkq_sb[:, j, bb, :],
                                    start=(i == 0),
                                    stop=(i == 3),
                                    skip_group_check=True,
                                )
                                i += 1
                        if c % BCHUNK == BCHUNK - 1:
                            p_sb = ppp.tile([P, 8 * BCHUNK], f16)
                            nc.scalar.activation(
                                p_sb, s_ps,
                                mybir.ActivationFunctionType.Exp,
                                scale=SCALE,
                            )
                            p_tiles.append(p_sb)
                            nc.tensor.matmul(
                                z_ps, ones_sb, p_sb,
                                start=(len(p_tiles) == 1),
                                stop=(len(p_tiles) == NBATCH),
                            )

                tc.no_sync_barrier()

                # Z normalization prep overlaps the phase-2 loads
                z64_sb = epi.tile([1, 64], f32)
                nc.vector.tensor_copy(z64_sb, z_ps)
                zc_sb = epi.tile([1, 32], f32)
                nc.vector.tensor_add(
                    zc_sb, z64_sb[:, 0:32], z64_sb[:, 32:64]
                )
                za_sb = epi.tile([1, 16], f32)
                nc.vector.tensor_add(
                    za_sb, zc_sb[:, 0:16], zc_sb[:, 16:32]
                )
                zsum_sb = epi.tile([1, H], f32)
                nc.vector.tensor_add(zsum_sb, za_sb[:, 0:8], za_sb[:, 8:16])
                zr_sb = epi.tile([1, H], f32)
                nc.vector.reciprocal(zr_sb, zsum_sb)
                zb_sb = epi.tile([P, H], f32)
                nc.gpsimd.partition_broadcast(zb_sb, zr_sb)
                # z128[p, j] = 1/Z[2j + (p >= 64)]
                z128_sb = epi.tile([P, 4], f32)
                zb_v = zb_sb[:, :].rearrange("p (j two) -> p j two", two=2)
                nc.vector.tensor_copy(z128_sb[0:64, :], zb_v[0:64, :, 0])
                nc.vector.tensor_copy(z128_sb[64:P, :], zb_v[64:P, :, 1])

                # ---- phase 2: plain loads, xa matmuls, weights ----
                for jj, (e, m0, nm) in enumerate(R_INSTS):
                    xr = xrp.tile([P, 4 * nm, D], f8)
                    eng[e].dma_start(
                        xr,
                        x_d[m0 * 512 : (m0 + nm) * 512, :].rearrange(
                            "(n p) d -> p n d", p=P
                        ),
                    )
                    if jj == 0:
                        eng[W_ENG["wv"]].dma_start(wv_sb, wv_d[:])
                        eng[W_ENG["bo2"]].dma_start(bo2_sb, bo2_d[:])
                    if jj == 1:
                        eng[W_ENG["wo"]].dma_start(wo_sb, wo_d[:])
                    for c in range(4 * nm):
                        cg = 4 * m0 + c  # global chunk
                        pt = p_tiles[cg // BCHUNK]
                        for k in range(4):
                            nc.tensor.matmul(
                                xa_ps[k],
                                xr[:, c, ts(k, P)],
                                pt[:, 8 * (cg % BCHUNK) :
                                   8 * (cg % BCHUNK) + 8],
                                start=(cg == 0),
                                stop=(cg == BCHUNK * NBATCH - 1),
                            )

                xa_sb = epi.tile([P, 4, H], f16)
                for k in range(4):
                    nc.vector.tensor_copy(xa_sb[:, k, :], xa_ps[k])

            with tc.tile_pool(name="pe1", bufs=1, space="PSUM") as pe1:
                # vt[p, j, c] = V_unnorm[head 2j+c][128j + p]
                vt_ps = pe1.tile([P, 4, 2], f32, name="vt", tag="vt")
                for j in range(4):
                    for k in range(4):
                        nc.tensor.matmul(
                            vt_ps[:, j, :],
                            wv_sb[:, k, ts(j, P)],
                            xa_sb[:, k, 2 * j : 2 * j + 2],
                            start=(k == 0),
                            stop=(k == 3),
                            skip_group_check=True,
                        )
                vt_sb = epi.tile([P, 4], f16)
                nc.vector.tensor_copy(vt_sb[0:64, :], vt_ps[0:64, :, 0])
                nc.vector.tensor_copy(vt_sb[64:P, :], vt_ps[64:P, :, 1])
                vtn_sb = epi.tile([P, 4], f16)
                nc.vector.tensor_mul(vtn_sb, vt_sb, z128_sb)

                # row128[p, j] = row[128j + p]
                row_ps = pe1.tile([P, 4], f32, name="row", tag="row")
                for j in range(4):
                    for k in range(4):
                        nc.tensor.matmul(
                            row_ps[:, j : j + 1],
                            wo_sb[:, k, ts(j, P)],
                            vtn_sb[:, k : k + 1],
                            start=(k == 0),
                            stop=(k == 3),
                            skip_group_check=True,
                        )
                row_sb = epi.tile([P, 4], f16)
                nc.vector.tensor_add(row_sb, row_ps, bo2_sb)

                # flatten [128, 4] -> [1, 512] through DRAM, then broadcast
                nc.scalar.dma_start(
                    row_d[0:1, :].rearrange("o (j p) -> (o p) j", p=P),
                    row_sb,
                )
                for e, r0, nr in W_INSTS:
                    eng[e].dma_start(
                        out_d[r0 : r0 + nr, :],
                        row_d[0:1, :].broadcast_to([nr, D]),
                    )

    if not nc.is_finalized():
        nc.finalize()
    return nc


def _get_nc():
    if "nc" not in _CACHE:
        _CACHE["nc"] = _build_bass()
    return _CACHE["nc"]


def _host_prep(inputs):
    poi = np.asarray(inputs["poi_data"], np.float32)
    wq1 = np.asarray(inputs["wq1"], np.float32)
    bq1 = np.asarray(inputs["bq1"], np.float32)
    wq2 = np.asarray(inputs["wq2"], np.float32)
    bq2 = np.asarray(inputs["bq2"], np.float32)
    wk = np.asarray(inputs["wk"], np.float32)

    q1 = (poi @ wq1 + bq1)[:, 0]  # [1683]
    q = q1 @ wq2 + bq2  # [512]
    qh = q.reshape(H, DH)
    wkq = np.stack(
        [wk[:, h * DH : (h + 1) * DH] @ qh[h] for h in range(H)], axis=1
    )  # [512, 8]
    return wkq.astype(np.float32)


def _make_in_maps(inputs):
    x = np.asarray(inputs["x"], np.float32)
    wv = np.asarray(inputs["wv"], np.float32)
    wo = np.asarray(inputs["wo"], np.float32)
    bv = np.asarray(inputs["bv"], np.float32).reshape(D)
    bo = np.asarray(inputs["bo"], np.float32).reshape(D)
    wkq = _host_prep(inputs)

    # wkq_sb[p, j, b, h] = wkq[256j + 2p + b, h]
    pidx = np.arange(P)
    wkq_l = np.zeros((2, 2, H, P), np.float16)
    for j in range(2):
        for bb in range(2):
            wkq_l[j, bb, :, :] = wkq[256 * j + 2 * pidx + bb, :].T
    wkq_l = np.ascontiguousarray(wkq_l.reshape(32, P).T)
    # wv_l[p, k, n] = wv[128k + p, n]
    wv_l = np.ascontiguousarray(
        wv.reshape(4, P, D).transpose(1, 0, 2)
    ).astype(np.float16)
    wo_l = np.ascontiguousarray(
        wo.reshape(4, P, D).transpose(1, 0, 2)
    ).astype(np.float16)
    bo2 = (bv @ wo + bo).reshape(D)
    bo2_l = np.ascontiguousarray(bo2.reshape(4, P).T).astype(np.float16)

    x8 = x.astype(ml_dtypes.float8_e4m3)

    return [
        {
            "x": np.ascontiguousarray(x8[b]),
            "wkq": wkq_l,
            "wv": wv_l,
            "wo": wo_l,
            "bo2": bo2_l,
        }
        for b in range(N_CORES)
    ]


def kernel(**inputs) -> np.ndarray:
    from concourse.bass_utils import run_bass_kernel_spmd

    nc = _get_nc()
    in_maps = _make_in_maps(inputs)
    res = run_bass_kernel_spmd(nc, in_maps, list(range(N_CORES)))
    out = np.stack(
        [np.asarray(res.results[b]["out"]) for b in range(N_CORES)], axis=0
    )
    return out.astype(np.float32)


# revision 29
# speedup vs baseline: 1.0344x; 1.0344x over previous
"""ExternalAttention kernel for Trainium2 (8 NeuronCores, batch-parallel).

Math (collapsed from the reference nn.Module):
  q = (poi_data @ wq1 + bq1)[:, 0] @ wq2 + bq2            # [512], shared
  per head h: wkq[:, h] = wk[:, 64h:64h+64] @ q[64h:64h+64] # [512, 8]
  scores = x @ wkq  (+ const per head -- cancels in softmax)
  A = softmax(scores / 8, axis=L)
  xa[h, :] = sum_l A[l, h] * x[l, :]                       # [8, 512]
  V[64h:64h+64] = xa[h] @ wv[:, 64h:64h+64]                # [512]
  row = (V / Z) @ wo + (bv @ wo + bo)                      # [512]
  out[b, l, :] = row_b  for every l.

Design (v6): x is cast to fp8(e4m3) on the host (rel-err gate is 2e-2,
measured ~5e-3).  Two-phase stream separated by a no_sync scheduler
barrier:

  Phase 1 (XBAR DMA-transposes, SP+Act): score tiles st[p,j,t](u16) =
    xpair[t, 128j+p]; fp8 d = 256j+2p+b.  Score matmuls contract
    (j, b) slices against a host-permuted wkq (plain-loaded first on
    SP; its single fence link delays only the first transpose), exp()
    emits tiny f16 p tiles [128, 32] that all stay live, and the
    softmax denominator accumulates on PE via a ones-vector matmul.

  Phase 2 (plain DMAs, SP+Act+Pool): raw fp8 row loads feeding the xa
    matmuls (lhsT = x chunk, rhs = p) plus the f16 epilogue weights.
    The Z-normalization prep is emitted right after the barrier so it
    overlaps the phase-2 loads.

The phases exist because the tile scheduler completion-fences
DmaTransposeAnt against InstDMACopy in both directions (any
engine/tensor), and transpose-loaded weights consumed as f16 matmul
operands mis-execute on the neuronx-cc/PJRT path; the barrier keeps
the scheduler from re-interleaving the classes.  The write tail
broadcasts the row from a [128, 512] f16 SBUF tile via stride-0
source APs on all three queues.  PSUM stays f32; f16 output rows are
upcast on the host.
"""

import os
import sys

import numpy as np

for _p in ("/opt/trn_rl_repo", "/opt/pypackages"):
    if os.path.isdir(_p) and _p not in sys.path:
        sys.path.append(_p)

import ml_dtypes

B, L, D = 8, 8192, 512
H, DH = 8, 64
P = 128
SCALE = 1.0 / np.sqrt(DH)  # 0.125
N_CORES = 8
BCHUNK = 8   # chunks per exp batch
NBATCH = 8   # p batches

# phase 1: (engine, macro0, n_macros), 1 macro = 512 tokens
T_INSTS = [
    ("sp", 0, 2), ("act", 2, 2), ("sp", 4, 4),
    ("act", 8, 4), ("sp", 12, 4),
]
# phase 2: plain fp8 row loads, (engine, macro0, n_macros)
R_INSTS = [
    ("pool", 0, 2), ("sp", 2, 2), ("act", 4, 2), ("pool", 6, 2),
    ("sp", 8, 2), ("act", 10, 2), ("pool", 12, 2), ("sp", 14, 2),
]
W_ENG = {"wv": "act", "wo": "pool", "bo2": "sp"}
# write tail: (engine, row0, nrows)
W_INSTS = [
    ("sp", 0, 2816), ("act", 2816, 2688), ("pool", 5504, 2688),
]

_CACHE = {}


def _build_bass():
    import concourse.bass as bass
    import concourse.tile as tile
    from concourse import mybir
    from concourse.bacc import Bacc

    f32 = mybir.dt.float32
    f16 = mybir.dt.float16
    f8 = mybir.dt.float8e4
    u16 = mybir.dt.uint16
    ts = bass.ts

    nc = Bacc(num_swdge_queues=4)
    x_d = nc.dram_tensor("x", [L, D], f8, kind="ExternalInput")
    wkq_d = nc.dram_tensor("wkq", [P, 32], f16, kind="ExternalInput")
    wv_d = nc.dram_tensor("wv", [P, 4, D], f16, kind="ExternalInput")
    wo_d = nc.dram_tensor("wo", [P, 4, D], f16, kind="ExternalInput")
    bo2_d = nc.dram_tensor("bo2", [P, 4], f16, kind="ExternalInput")
    row_d = nc.dram_tensor("row_scratch", [1, D], f16)
    out_d = nc.dram_tensor("out", [L, D], f16, kind="ExternalOutput")

    xu = x_d.bitcast(u16)  # [L, 256] pairs along d

    with tile.TileContext(nc) as tc:
        with (
            tc.tile_pool(name="consts", bufs=1) as consts,
            tc.tile_pool(name="xs", bufs=2) as xsp,
            tc.tile_pool(name="xr", bufs=6) as xrp,
            tc.tile_pool(name="pp", bufs=NBATCH) as ppp,
            tc.tile_pool(name="epi", bufs=1) as epi,
        ):
            eng = {"sp": nc.sync, "act": nc.scalar, "pool": nc.gpsimd}

            # wkq plain-loaded FIRST on SP (Act is busy with the exp
            # table load); only the first transpose fences behind it.
            wkq_sb = consts.tile([P, 2, 2, H], f16)
            nc.sync.dma_start(
                wkq_sb.rearrange("p j b h -> p (j b h)"), wkq_d[:, :]
            )
            ones_sb = consts.tile([P, 1], f16)
            nc.vector.memset(ones_sb, 1.0)
            warm = consts.tile([1, 8], f32)
            nc.vector.memset(warm, 0.0)
            warm_o = consts.tile([1, 8], f16)
            nc.scalar.activation(
                warm_o, warm, mybir.ActivationFunctionType.Exp, scale=1.0
            )

            wv_sb = consts.tile([P, 4, D], f16)
            wo_sb = consts.tile([P, 4, D], f16)
            bo2_sb = consts.tile([P, 4], f16)

            p_tiles = []

            with (
                tc.tile_pool(name="ps_acc", bufs=1, space="PSUM") as ps_acc,
                tc.tile_pool(name="ps_s", bufs=2, space="PSUM") as ps_s,
            ):
                z_ps = ps_acc.tile([1, 8 * BCHUNK], f32, name="zz", tag="zz")
                xa_ps = [
                    ps_acc.tile([P, H], f32, name=f"xa{k}", tag=f"xa{k}")
                    for k in range(4)
                ]

                # ---- phase 1: transposes, scores, exp, Z ----
                for e, m0, nm in T_INSTS:
                    tok0, ntok = m0 * 512, nm * 512
                    st = xsp.tile([P, 2, ntok], u16)
                    eng[e].dma_start(
                        st, xu[tok0 : tok0 + ntok, :], transpose=True
                    )
                    s8 = st.bitcast(f8).rearrange(
                        "p j (t b) -> p j t b", b=2
                    )
                    s_ps = None
                    for c in range(4 * nm):
                        if c % BCHUNK == 0:
                            s_ps = ps_s.tile([P, 8 * BCHUNK], f32)
                        col = 8 * (c % BCHUNK)
                        i = 0
                        for j in range(2):
                            for bb in range(2):
                                nc.tensor.matmul(
                                    s_ps[:, col : col + 8],
                                    s8[:, j, c * P : (c + 1) * P, bb],
                                    wkq_sb[:, j, bb, :],
                                    start=(i == 0),
                                    stop=(i == 3),
                                    skip_group_check=True,
                                )
                                i += 1
                        if c % BCHUNK == BCHUNK - 1:
                            p_sb = ppp.tile([P, 8 * BCHUNK], f16)
                            nc.scalar.activation(
                                p_sb, s_ps,
                                mybir.ActivationFunctionType.Exp,
                                scale=SCALE,
                            )
                            p_tiles.append(p_sb)
                            nc.tensor.matmul(
                                z_ps, ones_sb, p_sb,
                                start=(len(p_tiles) == 1),
                                stop=(len(p_tiles) == NBATCH),
                            )

                tc.no_sync_barrier()

                # Z normalization prep overlaps the phase-2 loads
                z64_sb = epi.tile([1, 64], f32)
                nc.vector.tensor_copy(z64_sb, z_ps)
                zc_sb = epi.tile([1, 32], f32)
                nc.vector.tensor_add(
                    zc_sb, z64_sb[:, 0:32], z64_sb[:, 32:64]
                )
                za_sb = epi.tile([1, 16], f32)
                nc.vector.tensor_add(
                    za_sb, zc_sb[:, 0:16], zc_sb[:, 16:32]
                )
                zsum_sb = epi.tile([1, H], f32)
                nc.vector.tensor_add(zsum_sb, za_sb[:, 0:8], za_sb[:, 8:16])
                zr_sb = epi.tile([1, H], f32)
                nc.vector.reciprocal(zr_sb, zsum_sb)
                zb_sb = epi.tile([P, H], f32)
                nc.gpsimd.partition_broadcast(zb_sb, zr_sb)
                # z128[p, j] = 1/Z[2j + (p >= 64)]
                z128_sb = epi.tile([P, 4], f32)
                zb_v = zb_sb[:, :].rearrange("p (j two) -> p j two", two=2)
                nc.vector.tensor_copy(z128_sb[0:64, :], zb_v[0:64, :, 0])
                nc.vector.tensor_copy(z128_sb[64:P, :], zb_v[64:P, :, 1])

                # ---- phase 2: plain loads, xa matmuls, weights ----
                for jj, (e, m0, nm) in enumerate(R_INSTS):
                    xr = xrp.tile([P, 4 * nm, D], f8)
                    eng[e].dma_start(
                        xr,
                        x_d[m0 * 512 : (m0 + nm) * 512, :].rearrange(
                            "(n p) d -> p n d", p=P
                        ),
                    )
                    if jj == 0:
                        eng[W_ENG["wv"]].dma_start(wv_sb, wv_d[:])
                        eng[W_ENG["bo2"]].dma_start(bo2_sb, bo2_d[:])
                    if jj == 1:
                        eng[W_ENG["wo"]].dma_start(wo_sb, wo_d[:])
                    for c in range(4 * nm):
                        cg = 4 * m0 + c  # global chunk
                        pt = p_tiles[cg // BCHUNK]
                        for k in range(4):
                            nc.tensor.matmul(
                                xa_ps[k],
                                xr[:, c, ts(k, P)],
                                pt[:, 8 * (cg % BCHUNK) :
                                   8 * (cg % BCHUNK) + 8],
                                start=(cg == 0),
                                stop=(cg == BCHUNK * NBATCH - 1),
                            )

                xa_sb = epi.tile([P, 4, H], f16)
                for k in range(4):
                    nc.vector.tensor_copy(xa_sb[:, k, :], xa_ps[k])

            with tc.tile_pool(name="pe1", bufs=1, space="PSUM") as pe1:
                # vt[p, j, c] = V_unnorm[head 2j+c][128j + p]
                vt_ps = pe1.tile([P, 4, 2], f32, name="vt", tag="vt")
                for j in range(4):
                    for k in range(4):
                        nc.tensor.matmul(
                            vt_ps[:, j, :],
                            wv_sb[:, k, ts(j, P)],
                            xa_sb[:, k, 2 * j : 2 * j + 2],
                            start=(k == 0),
                            stop=(k == 3),
                            skip_group_check=True,
                        )
                vt_sb = epi.tile([P, 4], f16)
                nc.vector.tensor_copy(vt_sb[0:64, :], vt_ps[0:64, :, 0])
                nc.vector.tensor_copy(vt_sb[64:P, :], vt_ps[64:P, :, 1])
                vtn_sb = epi.tile([P, 4], f16)
                nc.vector.tensor_mul(vtn_sb, vt_sb, z128_sb)

                # row128[p, j] = row[128j + p]
                row_ps = pe1.tile([P, 4], f32, name="row", tag="row")
                for j in range(4):
                    for k in range(4):
                        nc.tensor.matmul(
                            row_ps[:, j : j + 1],
                            wo_sb[:, k, ts(j, P)],
                            vtn_sb[:, k : k + 1],
                            start=(k == 0),
                            stop=(k == 3),
                            skip_group_check=True,
                        )
                row_sb = epi.tile([P, 4], f16)
                nc.vector.tensor_add(row_sb, row_ps, bo2_sb)

                # flatten [128, 4] -> [1, 512] through DRAM, then broadcast
                nc.scalar.dma_start(
                    row_d[0:1, :].rearrange("o (j p) -> (o p) j", p=P),
                    row_sb,
                )
                for e, r0, nr in W_INSTS:
                    eng[e].dma_start(
                        out_d[r0 : r0 + nr, :],
                        row_d[0:1, :].broadcast_to([nr, D]),
                    )

    if not nc.is_finalized():
        nc.finalize()
    return nc


def _get_nc():
    if "nc" not in _CACHE:
        _CACHE["nc"] = _build_bass()
    return _CACHE["nc"]


def _host_prep(inputs):
    poi = np.asarray(inputs["poi_data"], np.float32)
    wq1 = np.asarray(inputs["wq1"], np.float32)
    bq1 = np.asarray(inputs["bq1"], np.float32)
    wq2 = np.asarray(inputs["wq2"], np.float32)
    bq2 = np.asarray(inputs["bq2"], np.float32)
    wk = np.asarray(inputs["wk"], np.float32)

    q1 = (poi @ wq1 + bq1)[:, 0]  # [1683]
    q = q1 @ wq2 + bq2  # [512]
    qh = q.reshape(H, DH)
    wkq = np.stack(
        [wk[:, h * DH : (h + 1) * DH] @ qh[h] for h in range(H)], axis=1
    )  # [512, 8]
    return wkq.astype(np.float32)


def _make_in_maps(inputs):
    x = np.asarray(inputs["x"], np.float32)
    wv = np.asarray(inputs["wv"], np.float32)
    wo = np.asarray(inputs["wo"], np.float32)
    bv = np.asarray(inputs["bv"], np.float32).reshape(D)
    bo = np.asarray(inputs["bo"], np.float32).reshape(D)
    wkq = _host_prep(inputs)

    # wkq_sb[p, j, b, h] = wkq[256j + 2p + b, h]
    pidx = np.arange(P)
    wkq_l = np.zeros((2, 2, H, P), np.float16)
    for j in range(2):
        for bb in range(2):
            wkq_l[j, bb, :, :] = wkq[256 * j + 2 * pidx + bb, :].T
    wkq_l = np.ascontiguousarray(wkq_l.reshape(32, P).T)
    # wv_l[p, k, n] = wv[128k + p, n]
    wv_l = np.ascontiguousarray(
        wv.reshape(4, P, D).transpose(1, 0, 2)
    ).astype(np.float16)
    wo_l = np.ascontiguousarray(
        wo.reshape(4, P, D).transpose(1, 0, 2)
    ).astype(np.float16)
    bo2 = (bv @ wo + bo).reshape(D)
    bo2_l = np.ascontiguousarray(bo2.reshape(4, P).T).astype(np.float16)

    x8 = x.astype(ml_dtypes.float8_e4m3)

    return [
        {
            "x": np.ascontiguousarray(x8[b]),
            "wkq": wkq_l,
            "wv": wv_l,
            "wo": wo_l,
            "bo2": bo2_l,
        }
        for b in range(N_CORES)
    ]


def kernel(**inputs) -> np.ndarray:
    from concourse.bass_utils import run_bass_kernel_spmd

    nc = _get_nc()
    in_maps = _make_in_maps(inputs)
    res = run_bass_kernel_spmd(nc, in_maps, list(range(N_CORES)))
    out = np.stack(
        [np.asarray(res.results[b]["out"]) for b in range(N_CORES)], axis=0
    )
    return out.astype(np.float32)
